# revision 1
# baseline (speedup 1.0000x reference)
"""Trainium2 Bass kernel for nn_LinearPPI (block-sparse gene-gene message passing).

Computation (reference):
    out[b, 8*g_out + o] = sum_{n: block_out[n]=g_out} sum_i x[b, 8*block_in[n] + i] * w[n, i, o]
    out += x   (residual)

Strategy:
  - Blocks sorted by destination gene; destination genes sharded over 8 cores
    (edge/expert parallel, no collectives needed).
  - Per core, genes are paired into "quads" of QG=2 genes whose incoming-block
    counts sum close to a multiple of 16 (minimal padding).
  - Work is a stream of "windows": 16 x-slabs (one slab = 8 rows of x^T for
    one source gene = [8, 128]) stacked to a [128, 128] tile, plus a
    scattered weight tile.  Flipped-operand matmul per window:
        psum[:, f0:f0+ww] (+)= xwin.T @ wtile     (xwin = STATIONARY lhsT,
                                                   wtile = MOVING rhs)
    so each window costs only ww (8 or 16) PE columns, and output lands
    batch-major: psum[128 batch, 16] per quad -> PSUM holds all 250 quads
    of a core densely (16 half-bank tiles of [128, 256]).
  - Windows are PURE (all 16 slabs target one gene; w part [128, 8], fully
    dense) or MIX (both genes; w part [128, 16], half zeros).  A per-rank
    core-uniform pattern (nMIX, nG0, nG1) maximizes pure windows subject to
    every core's per-gene slab counts fitting; ~75% of windows are pure,
    which nearly halves the streamed weight bytes.
  - The whole stream (gathered x slabs + scattered weights) is fp8 e3m4,
    built on the host.  x is pre-scaled by SX=2 and w by SW=32 so both live
    in e3m4's normal range (~0.9% rms quantization); the decode divides by
    SX*SW.  The residual is NOT streamed: it is added exactly (f32) on the
    host, which takes residual precision out of the fp8 error budget.
  - The x-slab gather is done on the host (indices are known at trace time),
    producing a sequential HBM stream -> all device DMAs are large and
    contiguous (memory-bound regime; ~18.3 MB/core at ~360 GB/s dominates).
  - The per-core window schedule is identical across cores (rank-sorted
    window-count maxima + zero-padding) so a single SPMD program serves all
    8 cores; per-core variation lives only in the streamed data.
  - Per half-bank psum tile: one DVE copy to SBUF fp16 + one out-DMA issued
    from the ACT/Pool queues (never the SP queue that feeds stream chunks),
    overlapping the stream; out is a dense [128, ~4000] fp16 batch-major
    matrix, so the host decode is a cheap column permutation + residual add.
"""

import math
import numpy as np
import ml_dtypes

import concourse.bass as bass
import concourse.bacc as bacc
import concourse.mybir as mybir
from concourse.tile import TileContext
from concourse.bass_utils import run_bass_kernel_spmd


class Cfg:
    def __init__(self, G=4000, B=8, BATCH=128, NCORES=8, chunk=48, qg=2):
        assert G % NCORES == 0
        self.G, self.B, self.BATCH, self.NCORES = G, B, BATCH, NCORES
        self.GPC = G // NCORES            # genes per core
        self.QG = qg                      # genes per quad (M = QG*B)
        assert self.GPC % self.QG == 0
        self.NQ = self.GPC // self.QG     # quads per core
        self.NBANKS = math.ceil(self.NQ / 16)
        self.SLOTS = 16                   # slabs per window (K = 128)
        self.CH = chunk                   # windows per DMA chunk
        self.SX = 2.0                     # x pre-scale (keeps e3m4 normal)
        self.SW = 32.0                    # w pre-scale
        self.stream_np = ml_dtypes.float8_e3m4
        self.stream_dt = mybir.dt.float8e3
        self.out_np = np.float16
        self.out_dt = mybir.dt.float16


def _pack_host(cfg, x, w, block_in, block_out):
    """Sort/shard/pad on the host. Returns (in_maps, w_sched, decode_quads)."""
    G, B, BATCH, NC = cfg.G, cfg.B, cfg.BATCH, cfg.NCORES

    src = np.asarray(block_in, dtype=np.int64)
    dst = np.asarray(block_out, dtype=np.int64)
    w_full = np.asarray(w, dtype=np.float32) * cfg.SW

    order = np.argsort(dst, kind="stable")
    src_s = src[order]
    w_s = np.ascontiguousarray(w_full[order]).astype(cfg.stream_np)
    counts = np.bincount(dst, minlength=G)
    starts = np.zeros(G + 1, dtype=np.int64)
    np.cumsum(counts, out=starts[1:])

    # x^T slabs: xslab[g] = x[:, 8g:8g+8].T  -> [G, 8, BATCH]
    xslab = np.ascontiguousarray((np.asarray(x, dtype=np.float32) * cfg.SX).T
                                 .reshape(G, B, BATCH)).astype(cfg.stream_np)

    # --- balanced gene->core assignment (snake over count-sorted genes) ---
    order_g = np.argsort(-counts, kind="stable")
    core_of = np.empty(G, dtype=np.int64)
    for r in range(0, G, 2 * NC):
        blk = order_g[r : r + 2 * NC]
        pat = list(range(NC)) + list(range(NC - 1, -1, -1))
        for i, g in enumerate(blk):
            core_of[g] = pat[i]

    # --- per-core quad packing: target sums that are multiples of SLOTS ---
    per_core = []
    for c in range(NC):
        genes = np.where(core_of == c)[0]  # this core's genes
        pool = sorted(genes.tolist(), key=lambda g: -counts[g])
        quads = []
        for _ in range(cfg.NQ):
            q = [pool.pop(0)]                       # largest remaining
            while pool and len(q) < cfg.QG - 1:     # middle picks: big/small mix
                q.append(pool.pop(0) if len(q) % 2 else pool.pop(-1))
            if pool and len(q) < cfg.QG:
                s3 = sum(int(counts[g]) for g in q)
                # last pick: minimize padding to the next multiple of SLOTS
                best_i = min(range(len(pool)),
                             key=lambda i: (-(s3 + int(counts[pool[i]])))
                             % cfg.SLOTS)
                q.append(pool.pop(best_i))
            q.sort()
            quads.append(q)
        assert not pool
        q_slabs = np.array([sum(int(counts[g]) for g in q) for q in quads])
        q_wins = np.ceil(q_slabs / cfg.SLOTS).astype(np.int64)
        q_wins = np.maximum(q_wins, 1)
        # descending window-count order (biggest quads first); the tail of
        # the stream is then 1-window quads whose small psum tiles close
        # quickly, keeping the trailing copy-out chains short.
        rank = np.argsort(-q_wins, kind="stable")
        per_core.append(([quads[j] for j in rank], q_wins[rank]))

    # --- per-rank core-uniform window patterns ----------------------------
    # A window is PURE (one target gene, 8-wide w part) or MIX (both genes,
    # 16-wide).  Per rank pick (nMIX, nG0, nG1) maximizing pure windows while
    # every core's per-gene slab counts still fit:
    #   spill = max(0, a - 16*nG0) + max(0, b - 16*nG1) <= 16*nMIX
    w_rank = np.max(np.stack([pc[1] for pc in per_core]), axis=0)
    ab = np.zeros((NC, cfg.NQ, 2), dtype=np.int64)
    for c in range(NC):
        quads_r, _ = per_core[c]
        for j in range(cfg.NQ):
            ab[c, j, 0] = counts[quads_r[j][0]]
            ab[c, j, 1] = counts[quads_r[j][1]]
    # window descriptor per rank: list of types (2=MIX first, then 0=G0, 1=G1)
    win_types = []          # flattened [w_tot] list of (rank, type)
    rank_first = []         # first window index of each rank
    for j in range(cfg.NQ):
        W = int(w_rank[j])
        best = (0, 0, W)
        bestscore = -1
        for nG0 in range(W + 1):
            for nG1 in range(W - nG0 + 1):
                nM = W - nG0 - nG1
                if (nG0 == 0 and nM == 0) or (nG1 == 0 and nM == 0):
                    continue  # a gene's psum cols would never be written
                ok = True
                for c in range(NC):
                    a, b = int(ab[c, j, 0]), int(ab[c, j, 1])
                    if max(0, a - 16 * nG0) + max(0, b - 16 * nG1) > 16 * nM:
                        ok = False
                        break
                if ok and nG0 + nG1 > bestscore:
                    bestscore = nG0 + nG1
                    best = (nG0, nG1, nM)
        nG0, nG1, nM = best
        rank_first.append(len(win_types))
        win_types += [(j, 2)] * nM + [(j, 0)] * nG0 + [(j, 1)] * nG1
    w_tot = len(win_types)
    widths = np.array([BATCH + (16 if t == 2 else 8) for _, t in win_types])

    # chunk geometry (uniform across cores)
    bounds = list(range(0, w_tot, cfg.CH)) + [w_tot]
    n_chunks = len(bounds) - 1
    chunk_w = []            # per-chunk total width
    win_chunk = np.zeros(w_tot, dtype=np.int64)
    win_off = np.zeros(w_tot, dtype=np.int64)   # col offset within its chunk
    for cch in range(n_chunks):
        t0, t1 = bounds[cch], bounds[cch + 1]
        off = 0
        for t in range(t0, t1):
            win_chunk[t] = cch
            win_off[t] = off
            off += int(widths[t])
        chunk_w.append(off)
    maxW = max(chunk_w)

    sched = {
        "win_types": win_types, "rank_first": rank_first,
        "win_off": win_off, "chunk_w": chunk_w, "n_chunks": n_chunks,
        "maxW": maxW, "w_tot": w_tot, "bounds": bounds,
    }

    # --- build per-core streams -------------------------------------------
    in_maps = []
    decode_quads = []
    SL = cfg.SLOTS
    for c in range(NC):
        quads_r, _ = per_core[c]
        slab_gene = np.full((w_tot, SL), -1, dtype=np.int64)
        blk_ids, blk_pos_t, blk_pos_s, blk_rel = [], [], [], []
        for j in range(cfg.NQ):
            t0 = rank_first[j]
            t1 = rank_first[j + 1] if j + 1 < cfg.NQ else w_tot
            wmix = [t for t in range(t0, t1) if win_types[t][1] == 2]
            wpure = ([t for t in range(t0, t1) if win_types[t][1] == 0],
                     [t for t in range(t0, t1) if win_types[t][1] == 1])
            mix_cur = 0
            for r, g in enumerate(quads_r[j]):
                s0, n = int(starts[g]), int(counts[g])
                ids = np.arange(s0, s0 + n)
                cap = SL * len(wpure[r])
                take = min(n, cap)
                if take:
                    i = np.arange(take)
                    blk_ids.append(ids[:take])
                    blk_pos_t.append(np.array([wpure[r][k // SL] for k in i]))
                    blk_pos_s.append(i % SL)
                    blk_rel.append(np.full(take, r, dtype=np.int64))
                if take < n:
                    sp = n - take
                    i = mix_cur + np.arange(sp)
                    blk_ids.append(ids[take:])
                    blk_pos_t.append(np.array([wmix[k // SL] for k in i]))
                    blk_pos_s.append(i % SL)
                    blk_rel.append(np.full(sp, r, dtype=np.int64))
                    mix_cur += sp
        blk_ids = np.concatenate(blk_ids)
        blk_pos_t = np.concatenate(blk_pos_t)
        blk_pos_s = np.concatenate(blk_pos_s)
        blk_rel = np.concatenate(blk_rel)
        slab_gene[blk_pos_t, blk_pos_s] = src_s[blk_ids]

        # x slabs: [W, 128, BATCH]
        xg = np.zeros((w_tot, SL, B, BATCH), dtype=cfg.stream_np)
        m = slab_gene >= 0
        xg[m] = xslab[slab_gene[m]]
        xg = xg.reshape(w_tot, SL * B, BATCH)

        # scattered weights: [W, 128, 16] (col 8*rel; pure windows later
        # sliced to their 8 relevant columns)
        wg5 = np.zeros((w_tot, SL, B, cfg.QG, B), dtype=cfg.stream_np)
        wg5[blk_pos_t, blk_pos_s, :, blk_rel, :] = w_s[blk_ids]
        wg = wg5.reshape(w_tot, SL * B, cfg.QG * B)

        # chunk-major variable-width stream
        st = np.zeros((n_chunks * SL * B, maxW), dtype=cfg.stream_np)
        for cch in range(n_chunks):
            t0, t1 = bounds[cch], bounds[cch + 1]
            rows = slice(cch * 128, cch * 128 + 128)
            for t in range(t0, t1):
                off = int(win_off[t])
                typ = win_types[t][1]
                st[rows, off : off + BATCH] = xg[t]
                if typ == 2:
                    st[rows, off + BATCH : off + BATCH + 16] = wg[t]
                elif typ == 0:
                    st[rows, off + BATCH : off + BATCH + 8] = wg[t][:, 0:8]
                else:
                    st[rows, off + BATCH : off + BATCH + 8] = wg[t][:, 8:16]

        in_maps.append({"st": st})
        decode_quads.append(quads_r)

    return in_maps, sched, decode_quads


def _build_nc(cfg, sched):
    """Trace the (core-uniform) Bass program.

    Flipped-operand matmul: the gathered x window [128, 128] is the
    STATIONARY operand (lhsT), the scattered weight tile ([128, 8] pure /
    [128, 16] mixed) is the MOVING operand (rhs).  Each window then costs
    only 8-16 PE columns, and the output lands batch-major:
    psum[128 batch, 16] per quad, so PSUM holds all quads densely.
    """
    win_types = sched["win_types"]
    rank_first = sched["rank_first"]
    win_off = sched["win_off"]
    chunk_w = sched["chunk_w"]
    n_chunks = sched["n_chunks"]
    w_tot = sched["w_tot"]

    nc = bacc.Bacc("TRN2")
    st = nc.dram_tensor("st", [n_chunks * 128, sched["maxW"]], cfg.stream_dt,
                        kind="ExternalInput")
    NB = -(-cfg.NQ // 32)             # psum banks (32 quads per bank)
    out = nc.dram_tensor("out", [128, NB * 512 + 512], cfg.out_dt,
                         kind="ExternalOutput")

    CH = cfg.CH
    NW = cfg.BATCH            # x width per window (128)
    QW = cfg.QG * cfg.B       # psum region width per quad (16)
    QPT = 256 // QW           # quads per psum tile (16)
    NQT = -(-cfg.NQ // QPT)   # psum tiles (half-banks)
    rank_last = [rank_first[j + 1] - 1 if j + 1 < cfg.NQ else w_tot - 1
                 for j in range(cfg.NQ)]
    bounds = sched["bounds"]
    chunk_of = {bounds[c]: c for c in range(n_chunks)}

    with TileContext(nc) as tc:
        with (
            tc.tile_pool(name="stp", bufs=4) as stp,
            tc.tile_pool(name="psp", bufs=6, space="PSUM") as psp,
            tc.tile_pool(name="outp", bufs=8) as outp,
        ):
            st_t = None
            for qt in range(NQT):
                j0, j1 = qt * QPT, min(qt * QPT + QPT, cfg.NQ)
                fw = QW * (j1 - j0)   # used width of this psum tile
                ps = psp.tile([128, 256], mybir.dt.float32)
                for j in range(j0, j1):
                    f0 = QW * (j - j0)
                    t0, t1 = rank_first[j], rank_last[j] + 1
                    for t in range(t0, t1):
                        if t in chunk_of:
                            c = chunk_of[t]
                            st_t = stp.tile([128, chunk_w[c]], cfg.stream_dt)
                            nc.sync.dma_start(
                                out=st_t[:, :],
                                in_=st[c * 128 : (c + 1) * 128, : chunk_w[c]])
                        off = int(win_off[t])
                        typ = win_types[t][1]
                        o0, ww = (0, 16) if typ == 2 else (8 * typ, 8)
                        # start only on the quad's first window: it marks the
                        # whole psum zero-region pending, so later windows
                        # overwrite-on-first-touch per byte range (correct
                        # even when pure G0/G1 windows touch disjoint cols).
                        nc.tensor.matmul(
                            ps[:, f0 + o0 : f0 + o0 + ww],
                            st_t[:, off : off + NW],
                            st_t[:, off + NW : off + NW + ww],
                            start=(t == t0),
                            stop=(t == t1 - 1),
                            tile_position=(0, 0),
                            skip_group_check=True,
                        )
                # per-half-bank copy-out overlaps the stream (own psum tile
                # => fine-grained deps); out-DMAs issue away from the SP
                # queue so their sem waits never stall the stream chunks.
                # The last two tiles close right at stream end, so their
                # copy+DMA chains run on disjoint engines to overlap.
                # final tile: pad the DMA read to 256 cols so its runs are
                # >=512B (avoids the 2x small-element penalty on the critical
                # trailing chain); the copy still moves only the used cols.
                dw = 256 if qt == NQT - 1 else fw
                ot = outp.tile([128, dw], cfg.out_dt)
                nc.vector.tensor_copy(out=ot[:, :fw], in_=ps[:, :fw])
                eng = (nc.sync if qt >= NQT - 2
                       else nc.scalar if qt % 2 == 0
                       else nc.gpsimd)
                eng.dma_start(
                    out=out[:, qt * 256 : qt * 256 + dw], in_=ot[:, :])
    if not nc.is_finalized():
        nc.finalize()
    return nc


def _decode(cfg, results, decode_quads):
    G, B, BATCH = cfg.G, cfg.B, cfg.BATCH
    inv_s = 1.0 / (cfg.SX * cfg.SW)
    QW = cfg.QG * B
    out = np.empty((BATCH, G * B), dtype=np.float32)
    for c in range(cfg.NCORES):
        res = np.asarray(results[c]["out"], dtype=np.float32)
        for j in range(cfg.NQ):
            f0 = QW * j
            blockv = res[:, f0 : f0 + QW]          # [batch, QG*8]
            for r, g in enumerate(decode_quads[c][j]):
                out[:, 8 * g : 8 * g + 8] = blockv[:, 8 * r : 8 * r + 8]
    return out * inv_s


def _run(cfg, x, w, block_in, block_out, trace=False):
    in_maps, w_sched, decode_quads = _pack_host(cfg, x, w, block_in, block_out)
    nc = _build_nc(cfg, w_sched)
    r = run_bass_kernel_spmd(nc, in_maps, core_ids=list(range(cfg.NCORES)),
                             trace=trace)
    out = _decode(cfg, r.results, decode_quads)
    out = out + np.asarray(x, dtype=np.float32)   # exact residual on host
    return out, r


def kernel(x, w, block_in, block_out):
    cfg = Cfg()
    out, _ = _run(cfg, x, w, block_in, block_out, trace=False)
    return out



# revision 29
# speedup vs baseline: 1.0228x; 1.0228x over previous
"""Trainium2 Bass kernel for nn_LinearPPI (block-sparse gene-gene message passing).

Computation (reference):
    out[b, 8*g_out + o] = sum_{n: block_out[n]=g_out} sum_i x[b, 8*block_in[n] + i] * w[n, i, o]
    out += x   (residual)

Strategy:
  - Blocks sorted by destination gene; destination genes sharded over 8 cores
    (edge/expert parallel, no collectives needed).
  - Per core, genes are paired into "quads" of QG=2 genes whose incoming-block
    counts sum close to a multiple of 16 (minimal padding).
  - Work is a stream of "windows": 16 x-slabs (one slab = 8 rows of x^T for
    one source gene = [8, 128]) stacked to a [128, 128] tile, plus a
    scattered weight tile.  Flipped-operand matmul per window:
        psum[:, f0:f0+ww] (+)= xwin.T @ wtile     (xwin = STATIONARY lhsT,
                                                   wtile = MOVING rhs)
    so each window costs only ww (8 or 16) PE columns, and output lands
    batch-major: psum[128 batch, 16] per quad -> PSUM holds all 250 quads
    of a core densely (16 half-bank tiles of [128, 256]).
  - Windows are PURE (all 16 slabs target one gene; w part [128, 8], fully
    dense) or MIX (both genes; w part [128, 16], half zeros).  A per-rank
    core-uniform pattern (nMIX, nG0, nG1) maximizes pure windows subject to
    every core's per-gene slab counts fitting; ~75% of windows are pure,
    which nearly halves the streamed weight bytes.
  - The whole stream (gathered x slabs + scattered weights) is fp8 e3m4,
    built on the host.  x is pre-scaled by SX=2 and w by SW=32 so both live
    in e3m4's normal range (~0.9% rms quantization); the decode divides by
    SX*SW.  The residual is NOT streamed: it is added exactly (f32) on the
    host, which takes residual precision out of the fp8 error budget.
  - The x-slab gather is done on the host (indices are known at trace time),
    producing a sequential HBM stream -> all device DMAs are large and
    contiguous (memory-bound regime; ~18.3 MB/core at ~360 GB/s dominates).
  - The per-core window schedule is identical across cores (rank-sorted
    window-count maxima + zero-padding) so a single SPMD program serves all
    8 cores; per-core variation lives only in the streamed data.
  - Per half-bank psum tile: one DVE copy to SBUF fp16 + one out-DMA issued
    from the ACT/Pool queues (never the SP queue that feeds stream chunks),
    overlapping the stream; out is a dense [128, ~4000] fp16 batch-major
    matrix, so the host decode is a cheap column permutation + residual add.
"""

import math
import numpy as np
import ml_dtypes

import concourse.bass as bass
import concourse.bacc as bacc
import concourse.mybir as mybir
from concourse.tile import TileContext
from concourse.bass_utils import run_bass_kernel_spmd


class Cfg:
    def __init__(self, G=4000, B=8, BATCH=128, NCORES=8, chunk=48, qg=2):
        assert G % NCORES == 0
        self.G, self.B, self.BATCH, self.NCORES = G, B, BATCH, NCORES
        self.GPC = G // NCORES            # genes per core
        self.QG = qg                      # genes per quad (M = QG*B)
        assert self.GPC % self.QG == 0
        self.NQ = self.GPC // self.QG     # quads per core
        self.NBANKS = math.ceil(self.NQ / 16)
        self.SLOTS = 16                   # slabs per window (K = 128)
        self.CH = chunk                   # windows per DMA chunk
        self.SX = 2.0                     # x pre-scale (keeps e3m4 normal)
        self.SW = 32.0                    # w pre-scale
        self.stream_np = ml_dtypes.float8_e3m4
        self.stream_dt = mybir.dt.float8e3
        # fp8 output: psum holds SX*SW*out; the copy-out rescales by
        # 1/(SX*SW) so the stored value is the true block output (|.| < ~5,
        # inside e3m4 range) and decode needs no rescale.
        self.out_np = ml_dtypes.float8_e3m4
        self.out_dt = mybir.dt.float8e3


def _pack_host(cfg, x, w, block_in, block_out):
    """Sort/shard/pad on the host. Returns (in_maps, w_sched, decode_quads)."""
    G, B, BATCH, NC = cfg.G, cfg.B, cfg.BATCH, cfg.NCORES

    # --- dedup parallel edges: same (dst, src) blocks act as one with the
    # weights pre-summed (x @ w1 + x @ w2 == x @ (w1 + w2)) ------------------
    src0 = np.asarray(block_in, dtype=np.int64)
    dst0 = np.asarray(block_out, dtype=np.int64)
    w_full = np.asarray(w, dtype=np.float32) * cfg.SW
    key = dst0 * G + src0
    order = np.argsort(key, kind="stable")
    ks = key[order]
    first = np.ones(len(ks), dtype=bool)
    first[1:] = ks[1:] != ks[:-1]
    gstart = np.flatnonzero(first)
    w_sorted = w_full[order].reshape(len(ks), -1)
    w_uniq = np.add.reduceat(w_sorted, gstart, axis=0).reshape(-1, B, B)
    uk = ks[gstart]
    src_s = uk % G                        # per-edge src, sorted by (dst, src)
    dst_u = uk // G
    w_s = np.ascontiguousarray(w_uniq).astype(cfg.stream_np)
    counts = np.bincount(dst_u, minlength=G)
    starts = np.zeros(G + 1, dtype=np.int64)
    np.cumsum(counts, out=starts[1:])

    # x^T slabs: xslab[g] = x[:, 8g:8g+8].T  -> [G, 8, BATCH]
    xslab = np.ascontiguousarray((np.asarray(x, dtype=np.float32) * cfg.SX).T
                                 .reshape(G, B, BATCH)).astype(cfg.stream_np)

    # --- balanced gene->core assignment (snake over count-sorted genes) ---
    order_g = np.argsort(-counts, kind="stable")
    core_of = np.empty(G, dtype=np.int64)
    for r in range(0, G, 2 * NC):
        blk = order_g[r : r + 2 * NC]
        pat = list(range(NC)) + list(range(NC - 1, -1, -1))
        for i, g in enumerate(blk):
            core_of[g] = pat[i]

    # --- per-core quad pairing: joint overlap / alignment matching --------
    # A slab whose src feeds BOTH genes of a quad is placed once, in a MIX
    # window whose 16-wide w row carries both genes' blocks.  Pair genes by
    # greedy matching on  value = |src overlap| - pad  where pad is the
    # zero-slab padding of the union count up to the next multiple of 16
    # (the old packer spent the pairing freedom purely on pad).
    per_core = []
    for c in range(NC):
        genes = np.where(core_of == c)[0]
        n = len(genes)
        bits = np.zeros((n, G), np.float32)
        for i, g in enumerate(genes):
            bits[i, src_s[starts[g] : starts[g + 1]]] = 1.0
        ov = np.rint(bits @ bits.T).astype(np.int64)
        cnt = counts[genes]
        u = cnt[:, None] + cnt[None, :] - ov
        val = ov - ((-u) % cfg.SLOTS)
        np.fill_diagonal(val, -(10 ** 6))
        orderp = np.argsort(-val, axis=None)
        used = np.zeros(n, bool)
        quads, q_slabs, q_sh = [], [], []
        for f in orderp:
            i, j = divmod(int(f), n)
            if i < j and not used[i] and not used[j]:
                used[i] = used[j] = True
                quads.append(sorted((int(genes[i]), int(genes[j]))))
                q_slabs.append(int(u[i, j]))
                q_sh.append(int(ov[i, j]))
                if len(quads) == cfg.NQ:
                    break
        q_slabs = np.array(q_slabs)
        q_sh = np.array(q_sh)
        q_wins = np.maximum(np.ceil(q_slabs / cfg.SLOTS).astype(np.int64), 1)
        # descending window-count order (biggest quads first); the tail of
        # the stream is then 1-window quads whose small psum tiles close
        # quickly, keeping the trailing copy-out chains short.  Within equal
        # window counts, sort shared-heavy quads first: MIX windows are
        # forced per rank by the worst core, so aligning shared slabs to the
        # same ranks across cores keeps the MIX count (16-wide w) low.
        rank = np.argsort(-(q_wins * 64 + np.minimum(q_sh, 63)),
                          kind="stable")
        per_core.append(([quads[j] for j in rank], q_wins[rank]))

    # --- per-rank core-uniform window patterns ----------------------------
    # A window is PURE (one target gene, 8-wide w part) or MIX (both genes,
    # 16-wide).  Per rank pick (nMIX, nG0, nG1) maximizing pure windows while
    # every core's per-gene slab counts still fit; shared slabs (s) can only
    # live in MIX windows:
    #   max(0, a-16*nG0) + max(0, b-16*nG1) + s <= 16*nMIX
    w_rank = np.max(np.stack([pc[1] for pc in per_core]), axis=0)
    abs_ = np.zeros((NC, cfg.NQ, 3), dtype=np.int64)   # (a_only, b_only, s)
    for c in range(NC):
        quads_r, _ = per_core[c]
        for j in range(cfg.NQ):
            g0, g1 = quads_r[j]
            s0 = src_s[starts[g0] : starts[g0 + 1]]
            s1 = src_s[starts[g1] : starts[g1 + 1]]
            nsh = len(np.intersect1d(s0, s1, assume_unique=True))
            abs_[c, j] = (len(s0) - nsh, len(s1) - nsh, nsh)
    # Window types: 2 = MIX (16-wide w; any slot can serve either gene, and
    # ONLY these can hold shared slabs, whose 16-wide row carries both
    # genes' blocks), 3 = SPLIT (8-wide w like a pure window: slots 0-7 are
    # gene0 rows, 8-15 gene1 rows, executed as two K=64 matmuls at PE tile
    # rows 0/64), 0/1 = PURE.  Per rank minimize the number of 16-wide MIX
    # windows (the only ones paying the extra 8 w columns), using SPLITs as
    # the flexible spill absorber:
    #   need0 = max(0, a - 16*nG0 - 8*nSp), need1 = likewise
    #   need0 + need1 <= 16*nM16 - s   for every core
    win_types = []          # flattened [w_tot] list of (rank, type)
    rank_first = []         # first window index of each rank
    for j in range(cfg.NQ):
        W = int(w_rank[j])
        smax = int(abs_[:, j, 2].max())
        best = None
        for nM16 in range(-(-smax // 16), W + 1):
            for nSp in range(0, -1, -1):  # BISECT: splits disabled
                rest = W - nM16 - nSp
                for nG0 in range(rest + 1):
                    nG1 = rest - nG0
                    if nM16 == 0 and nSp == 0 and (nG0 == 0 or nG1 == 0):
                        continue  # a gene's psum cols would never be written
                    ok = True
                    for c in range(NC):
                        a, b, s = (int(abs_[c, j, 0]), int(abs_[c, j, 1]),
                                   int(abs_[c, j, 2]))
                        need = (max(0, a - 16 * nG0 - 8 * nSp)
                                + max(0, b - 16 * nG1 - 8 * nSp))
                        if need > 16 * nM16 - s:
                            ok = False
                            break
                    if ok:
                        best = (nM16, nSp, nG0, nG1)
                        break
                if best is not None:
                    break
            if best is not None:
                break
        assert best is not None, f"no feasible pattern for rank {j}"
        nM16, nSp, nG0, nG1 = best
        rank_first.append(len(win_types))
        win_types += ([(j, 2)] * nM16 + [(j, 3)] * nSp
                      + [(j, 0)] * nG0 + [(j, 1)] * nG1)
    w_tot = len(win_types)
    widths = np.array([BATCH + (16 if t == 2 else 8) for _, t in win_types])

    # chunk geometry (uniform across cores)
    bounds = list(range(0, w_tot, cfg.CH)) + [w_tot]
    n_chunks = len(bounds) - 1
    chunk_w = []            # per-chunk total width
    win_chunk = np.zeros(w_tot, dtype=np.int64)
    win_off = np.zeros(w_tot, dtype=np.int64)   # col offset within its chunk
    for cch in range(n_chunks):
        t0, t1 = bounds[cch], bounds[cch + 1]
        off = 0
        for t in range(t0, t1):
            win_chunk[t] = cch
            win_off[t] = off
            off += int(widths[t])
        chunk_w.append(off)
    maxW = max(chunk_w)

    sched = {
        "win_types": win_types, "rank_first": rank_first,
        "win_off": win_off, "chunk_w": chunk_w, "n_chunks": n_chunks,
        "maxW": maxW, "w_tot": w_tot, "bounds": bounds,
    }

    # --- build per-core streams -------------------------------------------
    in_maps = []
    decode_quads = []
    SL = cfg.SLOTS
    for c in range(NC):
        quads_r, _ = per_core[c]
        slab_gene = np.full((w_tot, SL), -1, dtype=np.int64)
        blk_ids, blk_pos_t, blk_pos_s, blk_rel = [], [], [], []
        for j in range(cfg.NQ):
            t0 = rank_first[j]
            t1 = rank_first[j + 1] if j + 1 < cfg.NQ else w_tot
            wmix = [t for t in range(t0, t1) if win_types[t][1] == 2]
            wsp = [t for t in range(t0, t1) if win_types[t][1] == 3]
            wpure = ([t for t in range(t0, t1) if win_types[t][1] == 0],
                     [t for t in range(t0, t1) if win_types[t][1] == 1])
            g0, g1 = quads_r[j]
            e0 = np.arange(starts[g0], starts[g0 + 1])
            e1 = np.arange(starts[g1], starts[g1 + 1])
            sh, i0, i1 = np.intersect1d(src_s[e0], src_s[e1],
                                        assume_unique=True,
                                        return_indices=True)
            # shared slabs first, one mix slot each, both w octets filled
            nsh = len(sh)
            if nsh:
                k = np.arange(nsh)
                tpos = np.array([wmix[v // SL] for v in k])
                spos = k % SL
                blk_ids.append(e0[i0]); blk_pos_t.append(tpos)
                blk_pos_s.append(spos)
                blk_rel.append(np.zeros(nsh, dtype=np.int64))
                blk_ids.append(e1[i1]); blk_pos_t.append(tpos)
                blk_pos_s.append(spos)
                blk_rel.append(np.ones(nsh, dtype=np.int64))
            mix_cur = nsh
            only = (np.delete(e0, i0), np.delete(e1, i1))
            for r in range(2):
                ids = only[r]
                n = len(ids)
                pos = 0
                # pure windows of this gene: 16 slots each
                take = min(n - pos, SL * len(wpure[r]))
                if take > 0:
                    i = np.arange(take)
                    blk_ids.append(ids[pos : pos + take])
                    blk_pos_t.append(np.array([wpure[r][k // SL] for k in i]))
                    blk_pos_s.append(i % SL)
                    blk_rel.append(np.full(take, r, dtype=np.int64))
                    pos += take
                # split windows: this gene's half, 8 slots, w octet 0
                take = min(n - pos, 8 * len(wsp))
                if take > 0:
                    i = np.arange(take)
                    blk_ids.append(ids[pos : pos + take])
                    blk_pos_t.append(np.array([wsp[k // 8] for k in i]))
                    blk_pos_s.append(i % 8 + 8 * r)
                    blk_rel.append(np.zeros(take, dtype=np.int64))
                    pos += take
                # leftover mix slots (after shared), octet r
                if pos < n:
                    sp = n - pos
                    i = mix_cur + np.arange(sp)
                    blk_ids.append(ids[pos:])
                    blk_pos_t.append(np.array([wmix[k // SL] for k in i]))
                    blk_pos_s.append(i % SL)
                    blk_rel.append(np.full(sp, r, dtype=np.int64))
                    mix_cur += sp
        blk_ids = np.concatenate(blk_ids)
        blk_pos_t = np.concatenate(blk_pos_t)
        blk_pos_s = np.concatenate(blk_pos_s)
        blk_rel = np.concatenate(blk_rel)
        slab_gene[blk_pos_t, blk_pos_s] = src_s[blk_ids]

        # x slabs: [W, 128, BATCH]
        xg = np.zeros((w_tot, SL, B, BATCH), dtype=cfg.stream_np)
        m = slab_gene >= 0
        xg[m] = xslab[slab_gene[m]]
        xg = xg.reshape(w_tot, SL * B, BATCH)

        # scattered weights: [W, 128, 16] (col 8*rel; pure windows later
        # sliced to their 8 relevant columns)
        wg5 = np.zeros((w_tot, SL, B, cfg.QG, B), dtype=cfg.stream_np)
        wg5[blk_pos_t, blk_pos_s, :, blk_rel, :] = w_s[blk_ids]
        wg = wg5.reshape(w_tot, SL * B, cfg.QG * B)

        # chunk-major variable-width stream
        st = np.zeros((n_chunks * SL * B, maxW), dtype=cfg.stream_np)
        for cch in range(n_chunks):
            t0, t1 = bounds[cch], bounds[cch + 1]
            rows = slice(cch * 128, cch * 128 + 128)
            for t in range(t0, t1):
                off = int(win_off[t])
                typ = win_types[t][1]
                st[rows, off : off + BATCH] = xg[t]
                if typ == 2:
                    st[rows, off + BATCH : off + BATCH + 16] = wg[t]
                elif typ == 1:
                    st[rows, off + BATCH : off + BATCH + 8] = wg[t][:, 8:16]
                else:   # pure g0 / split: w rows in octet 0
                    st[rows, off + BATCH : off + BATCH + 8] = wg[t][:, 0:8]

        in_maps.append({"st": st})
        decode_quads.append(quads_r)

    return in_maps, sched, decode_quads


def _build_nc(cfg, sched):
    """Trace the (core-uniform) Bass program.

    Flipped-operand matmul: the gathered x window [128, 128] is the
    STATIONARY operand (lhsT), the scattered weight tile ([128, 8] pure /
    [128, 16] mixed) is the MOVING operand (rhs).  Each window then costs
    only 8-16 PE columns, and the output lands batch-major:
    psum[128 batch, 16] per quad, so PSUM holds all quads densely.
    """
    win_types = sched["win_types"]
    rank_first = sched["rank_first"]
    win_off = sched["win_off"]
    chunk_w = sched["chunk_w"]
    n_chunks = sched["n_chunks"]
    w_tot = sched["w_tot"]

    nc = bacc.Bacc("TRN2")
    st = nc.dram_tensor("st", [n_chunks * 128, sched["maxW"]], cfg.stream_dt,
                        kind="ExternalInput")

    CH = cfg.CH
    NW = cfg.BATCH            # x width per window (128)
    QW = cfg.QG * cfg.B       # psum region width per quad (16)
    QPT = 256 // QW           # quads per psum tile (16)
    NQT = -(-cfg.NQ // QPT)   # psum tiles (half-banks)
    NPAIR = -(-NQT // 2)      # psum-tile pairs (one 512B-run out DMA each)
    out = nc.dram_tensor("out", [128, NPAIR * 512], cfg.out_dt,
                         kind="ExternalOutput")
    OS = 1.0 / (cfg.SX * cfg.SW)      # copy-out rescale into fp8 range
    rank_last = [rank_first[j + 1] - 1 if j + 1 < cfg.NQ else w_tot - 1
                 for j in range(cfg.NQ)]
    bounds = sched["bounds"]
    chunk_of = {bounds[c]: c for c in range(n_chunks)}

    with TileContext(nc) as tc:
        with (
            tc.tile_pool(name="stp", bufs=4) as stp,
            tc.tile_pool(name="psp", bufs=6, space="PSUM") as psp,
            tc.tile_pool(name="outp", bufs=8) as outp,
        ):
            st_t = None
            for qt in range(NQT):
                j0, j1 = qt * QPT, min(qt * QPT + QPT, cfg.NQ)
                fw = QW * (j1 - j0)   # used width of this psum tile
                ps = psp.tile([128, 256], mybir.dt.float32)
                for j in range(j0, j1):
                    f0 = QW * (j - j0)
                    t0, t1 = rank_first[j], rank_last[j] + 1
                    for t in range(t0, t1):
                        if t in chunk_of:
                            c = chunk_of[t]
                            st_t = stp.tile([128, chunk_w[c]], cfg.stream_dt)
                            nc.sync.dma_start(
                                out=st_t[:, :],
                                in_=st[c * 128 : (c + 1) * 128, : chunk_w[c]])
                        off = int(win_off[t])
                        typ = win_types[t][1]
                        # start only on the quad's first window: it marks the
                        # whole psum zero-region pending, so later windows
                        # overwrite-on-first-touch per byte range (correct
                        # even when pure G0/G1 windows touch disjoint cols).
                        if typ == 3:
                            # split window: slots 0-7 feed gene0, 8-15 gene1,
                            # sharing one 8-wide w block via two K=64 matmuls
                            # on PE tile rows 0 / 64.
                            for h in range(2):
                                nc.tensor.matmul(
                                    ps[:, f0 + 8 * h : f0 + 8 * h + 8],
                                    st_t[64 * h : 64 * h + 64,
                                         off : off + NW],
                                    st_t[64 * h : 64 * h + 64,
                                         off + NW : off + NW + 8],
                                    start=(t == t0),
                                    stop=(t == t1 - 1),
                                    tile_position=(64 * h, 0),
                                    skip_group_check=True,
                                )
                        else:
                            o0, ww = (0, 16) if typ == 2 else (8 * typ, 8)
                            nc.tensor.matmul(
                                ps[:, f0 + o0 : f0 + o0 + ww],
                                st_t[:, off : off + NW],
                                st_t[:, off + NW : off + NW + ww],
                                start=(t == t0),
                                stop=(t == t1 - 1),
                                tile_position=(0, 0),
                                skip_group_check=True,
                            )
                # per-half-bank copy-out overlaps the stream (own psum tile
                # => fine-grained deps).  Two tiles share one [128, 512] fp8
                # staging tile so every out DMA has >=512B runs (fp8 tiles
                # alone would be 256B and pay the 2x small-run penalty);
                # out-DMAs issue away from the SP queue so their sem waits
                # never stall the stream chunks.
                pt = qt // 2
                if qt % 2 == 0:
                    ot = outp.tile([128, 512], cfg.out_dt)
                nc.vector.tensor_scalar_mul(
                    out=ot[:, (qt % 2) * 256 : (qt % 2) * 256 + fw],
                    in0=ps[:, :fw], scalar1=OS)
                if qt % 2 == 1 or qt == NQT - 1:
                    eng = (nc.sync if pt == NPAIR - 1
                           else nc.scalar if pt % 2 == 0
                           else nc.gpsimd)
                    eng.dma_start(
                        out=out[:, pt * 512 : pt * 512 + 512], in_=ot[:, :])
    if not nc.is_finalized():
        nc.finalize()
    return nc


def _decode(cfg, results, decode_quads):
    G, B, BATCH = cfg.G, cfg.B, cfg.BATCH
    QW = cfg.QG * B
    out = np.empty((BATCH, G * B), dtype=np.float32)
    for c in range(cfg.NCORES):
        res = np.asarray(results[c]["out"]).astype(np.float32)
        for j in range(cfg.NQ):
            f0 = QW * j
            blockv = res[:, f0 : f0 + QW]          # [batch, QG*8]
            for r, g in enumerate(decode_quads[c][j]):
                out[:, 8 * g : 8 * g + 8] = blockv[:, 8 * r : 8 * r + 8]
    return out


def _run(cfg, x, w, block_in, block_out, trace=False):
    in_maps, w_sched, decode_quads = _pack_host(cfg, x, w, block_in, block_out)
    nc = _build_nc(cfg, w_sched)
    r = run_bass_kernel_spmd(nc, in_maps, core_ids=list(range(cfg.NCORES)),
                             trace=trace)
    out = _decode(cfg, r.results, decode_quads)
    out = out + np.asarray(x, dtype=np.float32)   # exact residual on host
    return out, r


def kernel(x, w, block_in, block_out):
    cfg = Cfg()
    out, _ = _run(cfg, x, w, block_in, block_out, trace=False)
    return out

